# revision 3
# baseline (speedup 1.0000x reference)
"""Causal multi-head attention kernel for 8 trn2 NeuronCores.

Problem: x[2,2048,1024], 16 heads of dim 64, causal softmax(q k^T / sqrt(1024)) v,
then output projection. Sharding: data-parallel over batch (4 cores per batch),
tensor-parallel over heads (4 heads per core). Each core produces a partial
output (its heads' contribution through Wout); the host sums the 4 partials per
batch and adds b_out.

Per-core device program (SPMD), v2 — cost-model-scheduled:
  - Same math/layout as v1: xT [d, n] bf16; qT/kT per head-PAIR (head A on
    partitions 0..63, head B on 64..127); v natural per (nb, pair) as
    [ones64|dataA64|ones64|dataB64] so the AV lhsT [ones|data] window makes
    the AV matmul also emit softmax row-sums on pO partitions 0..63;
    S^T pair via PE row-group tiling (two K=64 matmuls run concurrently);
    one ACT exp instruction per step covers both heads; block-causal skipping;
    normalize straight from PSUM via reciprocal_approx_fast + tensor_mul.
  - Head: input DMAs are issued immediately and split across queues (weights
    on the scalar queue, x in 5 column-chunks on sync) so bytes flow by ~3us
    instead of ~9; the v_all ones-memset runs on the idle gpsimd engine; a
    train of small garbage dummy matmuls keeps the PE active from ~0.3us so
    the HAM activity window promotes to K=8/8 during the DMA phase.
  - Body: the attention steps are paced by the ACT engine (exp is the
    per-step critical resource: ~1.1us vs ~0.64us of PE work per full step).
    Fillers (q/k/v projections, output projection) are split into ~430ns
    units (mid-PSUM-chain yields) and pulled by a static cost model that
    keeps the PE's cumulative time just under ACT's, so S-pair matmuls are
    never queued behind long filler chains and exp never starves. Hard
    dependencies (q/k chunks, v blocks before their AVs) are force-pulled.
  - Tail: for the last 4 row-blocks the Wout contraction is split by pair:
    pair-0 partials are computed right after pair-0's last normalize and
    parked in SBUF (f32); after the final normalize only 8 pair-1 matmuls +
    DVE add-casts + 4 gpsimd DMAs remain (~4us instead of ~20us, and the PE
    never idles long enough for the HAM to demote to K=4/8).
"""

import os

import numpy as np
import ml_dtypes

B, N, D, H = 2, 2048, 1024, 16
DH = D // H  # 64
SCALE = float(D) ** -0.5
NCORES = 8
HPC = 4  # heads per core
NP = 2  # head pairs per core
IC = 512  # i-chunk width
NB = N // 128  # 16 j blocks
NCP = N // IC  # 4 i-chunks
KT = D // 128  # 8 contraction tiles
VW = 256  # v cols per (nb, pair): ones(64) | dataA(64) | ones(64) | dataB(64)
DELAY = 3
NDUMMY = 55

_cached = {}
_last_results = None


def _build_program():
    import concourse.bacc as bacc
    import concourse.mybir as mybir
    import concourse.tile as tile

    f32 = mybir.dt.float32
    bf16 = mybir.dt.bfloat16
    EXP = mybir.ActivationFunctionType.Exp

    nc = bacc.Bacc()

    xb = nc.dram_tensor("xb", [D, N], bf16, kind="ExternalInput")  # x^T
    wq = nc.dram_tensor("wq", [D, HPC * DH], bf16, kind="ExternalInput")
    wk = nc.dram_tensor("wk", [D, HPC * DH], bf16, kind="ExternalInput")
    wv = nc.dram_tensor("wv", [D, HPC * DH], bf16, kind="ExternalInput")
    wo = nc.dram_tensor("wo", [HPC * DH, D], bf16, kind="ExternalInput")
    tri = nc.dram_tensor("tri", [128, 128], bf16, kind="ExternalInput")
    outp = nc.dram_tensor("outp", [N, D], bf16, kind="ExternalOutput")

    with tile.TileContext(nc) as tc:
        with (
            tc.tile_pool(name="const", bufs=1) as const_pool,
            tc.tile_pool(name="big", bufs=1) as big_pool,
            tc.tile_pool(name="pS", bufs=2, space="PSUM") as pS_pool,
            tc.tile_pool(name="pO", bufs=2, space="PSUM") as pO_pool,
            tc.tile_pool(name="pj", bufs=2, space="PSUM") as pj_pool,
            tc.tile_pool(name="att", bufs=5) as att_pool,
            tc.tile_pool(name="rec", bufs=4) as rec_pool,
            tc.tile_pool(name="osb", bufs=3) as osb_pool,
        ):
            # ---- tiles ----
            warm = const_pool.tile([1, 8], f32, name="warm", tag="warm")
            wa = const_pool.tile([128, 128], bf16, name="wa", tag="wa")
            wqa = const_pool.tile([128, KT * 256], bf16, name="wqa", tag="wqa")
            wka = const_pool.tile([128, KT * 256], bf16, name="wka", tag="wka")
            wva = const_pool.tile([128, KT * 256], bf16, name="wva", tag="wva")
            woa = const_pool.tile([128, NP * D], bf16, name="woa", tag="woa")
            tri_sb = const_pool.tile([128, 128], bf16, name="tri_sb", tag="tri_sb")
            xTall = big_pool.tile([128, KT * N], bf16, name="xTall", tag="xTall")
            xT = [xTall[:, N * r : N * (r + 1)] for r in range(KT)]
            xT4 = xTall.rearrange("p (r c) -> p r c", r=KT)
            v_all = big_pool.tile([128, NB * NP * VW], bf16, name="v_all", tag="v_all")
            va8 = v_all.rearrange("p (n g c) -> p n g c", n=NB, g=8)
            qT, kT_, OT = [], [], []
            for p in range(NP):
                qT.append(big_pool.tile([128, N], bf16, name=f"qT{p}", tag=f"qT{p}"))
                kT_.append(big_pool.tile([128, N], bf16, name=f"kT{p}", tag=f"kT{p}"))
                OT.append(big_pool.tile([128, N], bf16, name=f"OT{p}", tag=f"OT{p}"))
            # pair-0 partial out-projections for the last 4 row-blocks (f32)
            part = big_pool.tile([128, 8 * 512], f32, name="part", tag="part")

            wq_sb = [wqa[:, 256 * r : 256 * (r + 1)] for r in range(KT)]
            wk_sb = [wka[:, 256 * r : 256 * (r + 1)] for r in range(KT)]
            wv_sb = [wva[:, 256 * r : 256 * (r + 1)] for r in range(KT)]
            wo_sb = [woa[:, D * p : D * (p + 1)] for p in range(NP)]

            # ---- t=0: prime every queue ----
            # vector: tiny memsets the dummies + warm exp depend on
            nc.vector.memset(warm, 0.0)
            nc.vector.memset(wa, 0.0)
            # scalar: small + weight DMAs, ACT table warm-up between them
            nc.scalar.dma_start(out=tri_sb, in_=tri[:, :])
            nc.scalar.dma_start(
                out=wva, in_=wv[:, :].rearrange("(r p) c -> p r c", r=KT)
            )
            nc.scalar.activation(out=warm, in_=warm, func=EXP, scale=1.0)
            nc.scalar.dma_start(
                out=wka, in_=wk[:, :].rearrange("(r p) c -> p r c", r=KT)
            )
            nc.scalar.dma_start(
                out=wqa, in_=wq[:, :].rearrange("(r p) c -> p r c", r=KT)
            )
            nc.scalar.dma_start(
                out=woa, in_=wo[:, :].rearrange("(p q) c -> q p c", p=NP)
            )
            # sync: x column chunks, first-needed first
            xchunks = [(0, 128), (128, 512), (512, 1024), (1024, 1536), (1536, 2048)]
            for lo, hi in xchunks:
                nc.sync.dma_start(
                    out=xT4[:, :, lo:hi],
                    in_=xb[:, lo:hi].rearrange("(r p) c -> p r c", r=KT),
                )
            # gpsimd: the big ones-memset (data cols overwritten by vproj)
            nc.gpsimd.memset(v_all, 1.0)
            # tensor: garbage dummy matmuls keep the PE active so the HAM
            # promotes to K=8/8 while the input DMA streams
            for _ in range(NDUMMY):
                pw = pj_pool.tile([128, 512], f32, name="pw", tag="pj")
                nc.tensor.matmul(pw[:, 0:128], lhsT=wa, rhs=wa, start=True, stop=True)

            # ---- static DMA-arrival model (ns), for filler gating only ----
            ICACHE, ISS_SC, ISS_SY, LAT, BWR = 2600.0, 700.0, 1250.0, 1500.0, 0.33
            arrivals = []
            t = ICACHE
            for i, (nm, sz) in enumerate(
                [("tri", 32768), ("wv", 524288), ("wk", 524288), ("wq", 524288), ("wo", 524288)]
            ):
                t += ISS_SC + (600.0 if nm == "wk" else 0.0)  # warm exp before wk
                arrivals.append((t + LAT, nm, sz))
            t = ICACHE
            for (lo, hi) in xchunks:
                t += ISS_SY
                arrivals.append((t + LAT, f"x{hi}", (hi - lo) * 128 * KT * 2))
            arrivals.sort()
            dma_est = {}
            tsrv = 0.0
            for arr, nm, sz in arrivals:
                tsrv = max(tsrv, arr) + sz / BWR / 1000.0
                dma_est[nm] = tsrv

            def xgate(hi_col):
                for lo, hi in xchunks:
                    if hi >= hi_col:
                        return dma_est[f"x{hi}"]
                return dma_est["x2048"]

            # ---- scheduler state ----
            st = {"tPE": 300.0 + NDUMMY * 107.0, "tACT": 6500.0}
            exp_end = {}
            norm_done = {}

            def rate():
                return 1.2 if st["tPE"] < 8000.0 else 2.4

            def addPE(cols, gate=0.0):
                st["tPE"] = max(st["tPE"], gate) + cols / rate()

            # ---- unit streams ----
            # Each stream is a list of units (cols, gate_fn, emit_fn) plus a
            # chain-open flag so pj-ring (bufs=2) allocations never interleave
            # with two other open chains.
            class Stream:
                def __init__(self, units, opens_chain=False, n_chain=1):
                    self.units = units
                    self.i = 0
                    self.n_chain = n_chain  # units per pj chain

                def peek(self):
                    return self.units[self.i] if self.i < len(self.units) else None

                def mid_chain(self):
                    return self.i % self.n_chain != 0

                def pop(self):
                    u = self.units[self.i]
                    self.i += 1
                    return u

                def done(self):
                    return self.i >= len(self.units)

            def v_stream(nb):
                box = {}

                def emit(half, nb=nb, box=box):
                    def f():
                        if half == 0:
                            box["pv"] = pj_pool.tile(
                                [128, HPC * DH], f32, name="pv", tag="pj"
                            )
                        for r in range(4 * half, 4 * half + 4):
                            nc.tensor.matmul(
                                box["pv"],
                                lhsT=xT[r][:, 128 * nb : 128 * (nb + 1)],
                                rhs=wv_sb[r],
                                start=(r == 0),
                                stop=(r == KT - 1),
                            )
                        if half == 1:
                            pv4 = box["pv"].rearrange("p (h c) -> p h c", h=HPC)
                            nc.vector.tensor_copy(out=va8[:, nb, 1::2, :], in_=pv4)

                    return f

                g = max(xgate(128 * (nb + 1)), dma_est["wv"])
                return Stream(
                    [(1024.0, g, emit(0)), (1024.0, g, emit(1))], n_chain=2
                )

            def qk_stream(p, c, which):
                box = {}
                w_sb = wk_sb if which == "k" else wq_sb
                dst = kT_[p] if which == "k" else qT[p]
                sl = slice(IC * c, IC * (c + 1))

                def emit(j, p=p, box=box):
                    def f():
                        if j == 0:
                            box["pq"] = pj_pool.tile([128, IC], f32, name="pq", tag="pj")
                        for r in range(2 * j, 2 * j + 2):
                            nc.tensor.matmul(
                                box["pq"],
                                lhsT=w_sb[r][:, 128 * p : 128 * (p + 1)],
                                rhs=xT[r][:, sl],
                                start=(r == 0),
                                stop=(r == KT - 1),
                            )
                        if j == 3:
                            nc.vector.tensor_copy(out=dst[:, sl], in_=box["pq"])

                    return f

                g = max(xgate(IC * (c + 1)), dma_est["wk" if which == "k" else "wq"])
                return Stream([(1024.0, g, emit(j)) for j in range(4)], n_chain=4)

            def o_unit(nb, s):
                nsl = slice(128 * nb, 128 * (nb + 1))
                cp = nb // 4

                def gate():
                    return 0.0 if norm_done.get((cp, 0)) and norm_done.get((cp, 1)) else None

                def f():
                    po = pj_pool.tile([128, 512], f32, name="po", tag="pj")
                    for p in range(NP):
                        nc.tensor.matmul(
                            po,
                            lhsT=OT[p][:, nsl],
                            rhs=wo_sb[p][:, 512 * s : 512 * (s + 1)],
                            start=(p == 0),
                            stop=(p == NP - 1),
                        )
                    ob = osb_pool.tile([128, 512], bf16, name="ob", tag="osb")
                    nc.vector.tensor_copy(out=ob, in_=po)
                    nc.gpsimd.dma_start(out=outp[nsl, 512 * s : 512 * (s + 1)], in_=ob)

                return (1024.0, gate, f)

            def op0_unit(nb, s):
                nsl = slice(128 * nb, 128 * (nb + 1))
                off = 512 * (2 * (nb - 12) + s)

                def gate():
                    return 0.0 if norm_done.get((3, 0)) else None

                def f():
                    po = pj_pool.tile([128, 512], f32, name="po0", tag="pj")
                    nc.tensor.matmul(
                        po,
                        lhsT=OT[0][:, nsl],
                        rhs=wo_sb[0][:, 512 * s : 512 * (s + 1)],
                        start=True,
                        stop=True,
                    )
                    nc.vector.tensor_copy(out=part[:, off : off + 512], in_=po)

                return (512.0, gate, f)

            vstr = {nb: v_stream(nb) for nb in range(NB)}
            kstr = {(p, c): qk_stream(p, c, "k") for p in range(NP) for c in range(NCP)}
            qstr = {(p, c): qk_stream(p, c, "q") for p in range(NP) for c in range(NCP)}
            ostr = {
                nb: Stream([o_unit(nb, 0), o_unit(nb, 1)]) for nb in range(12)
            }
            op0str = {
                nb: Stream([op0_unit(nb, 0), op0_unit(nb, 1)]) for nb in range(12, 16)
            }

            # budget-pull priority order
            fillers = []
            for c in range(NCP):
                fillers += [kstr[(0, c)], qstr[(0, c)]]
                fillers += [vstr[nb] for nb in range(4 * c, 4 * c + 4)]
                fillers += [kstr[(1, c)], qstr[(1, c)]]
                if c >= 1:
                    fillers += [ostr[nb] for nb in range(4 * (c - 1), 4 * c)]
            fillers += [ostr[nb] for nb in range(8, 12)]
            fillers += [op0str[nb] for nb in range(12, 16)]

            # pj is a 2-slot ring: a new pj allocation while an earlier
            # multi-unit chain is still open would race. Enforce: at most one
            # open chain, and complete it before any other pj allocation.
            sched = {"open": None}

            def _emit_unit(s):
                cols, gate, emit = s.pop()
                g = gate() if callable(gate) else gate
                emit()
                addPE(cols, g or 0.0)
                sched["open"] = s if (not s.done() and s.mid_chain()) else None

            def force_pull(stream):
                if sched["open"] is not None and sched["open"] is not stream:
                    o = sched["open"]
                    while not o.done() and o.mid_chain():
                        _emit_unit(o)
                    sched["open"] = None
                while not stream.done():
                    _emit_unit(stream)

            def budget_pull():
                if sched["open"] is not None:
                    s = sched["open"]
                    cols, gate, emit = s.peek()
                    if st["tPE"] + cols / rate() > st["tACT"]:
                        return False
                    _emit_unit(s)
                    return True
                for s in fillers:
                    if s.done():
                        continue
                    cols, gate, emit = s.peek()
                    g = gate() if callable(gate) else gate
                    if g is None:
                        continue
                    if st["tPE"] < g - 400.0:
                        continue
                    if st["tPE"] + cols / rate() > st["tACT"]:
                        return False
                    _emit_unit(s)
                    return True
                return False

            # ---- attention ----
            pend = []

            def drain(n):
                while len(pend) > n:
                    pend.pop(0)()

            t_idx = 0
            for cp in range(NCP):
                for p in range(NP):
                    pO_A = pO_pool.tile([128, IC], f32, name=f"pOA{cp}{p}", tag="pO")
                    pO_B = pO_pool.tile([128, IC], f32, name=f"pOB{cp}{p}", tag="pO")
                    jmax = 4 * cp + 4
                    for jb in range(jmax):
                        o = max(0, 128 * jb - IC * cp)
                        jsl = slice(128 * jb, 128 * (jb + 1))
                        isl = slice(IC * cp + o, IC * (cp + 1))
                        # hard deps for this step's S and the AV it will drain
                        force_pull(kstr[(p, jb // 4)])
                        force_pull(qstr[(p, cp)])
                        for nb in range(jb + 1):
                            force_pull(vstr[nb])
                        # S^T pair: K=64 each, concurrent via row groups
                        st["tPE"] = max(st["tPE"], exp_end.get(t_idx - 2, 0.0))
                        pS = pS_pool.tile([128, 2 * IC], f32, name="pS", tag="pS")
                        pexp = att_pool.tile([128, 2 * IC], bf16, name="pexp", tag="pexp")
                        nc.tensor.matmul(
                            pS[:, o:IC],
                            lhsT=kT_[p][0:64, jsl],
                            rhs=qT[p][0:64, isl],
                            start=True,
                            stop=True,
                        )
                        nc.tensor.matmul(
                            pS[:, IC + o : 2 * IC],
                            lhsT=kT_[p][64:128, jsl],
                            rhs=qT[p][64:128, isl],
                            start=True,
                            stop=True,
                        )
                        addPE((IC - o) + 40.0)
                        # one exp for both heads: [128, 2, IC-o] strided AP
                        src = pS.rearrange("p (h w) -> p h w", h=2)[:, :, o:]
                        dst = pexp.rearrange("p (h w) -> p h w", h=2)[:, :, o:]
                        nc.scalar.activation(out=dst, in_=src, func=EXP, scale=SCALE)
                        st["tACT"] = max(st["tACT"], st["tPE"]) + 260.0 + 1.7 * (IC - o)
                        exp_end[t_idx] = st["tACT"]
                        if 128 * jb >= IC * cp:  # diagonal block: 0/1 mask
                            for half in range(2):
                                hb = IC * half
                                nc.vector.tensor_mul(
                                    pexp[:, hb + o : hb + o + 128],
                                    pexp[:, hb + o : hb + o + 128],
                                    tri_sb,
                                )

                        def av_unit(p=p, jb=jb, o=o, jmax=jmax, pO_A=pO_A, pO_B=pO_B, pexp=pexp):
                            vo = 2 * VW * jb + VW * p
                            nc.tensor.matmul(
                                pO_A[:, o:IC],
                                lhsT=v_all[:, vo : vo + 128],
                                rhs=pexp[:, o:IC],
                                start=(jb == 0),
                                stop=(jb == jmax - 1),
                                skip_group_check=True,
                            )
                            nc.tensor.matmul(
                                pO_B[:, o:IC],
                                lhsT=v_all[:, vo + 128 : vo + 256],
                                rhs=pexp[:, IC + o : 2 * IC],
                                start=(jb == 0),
                                stop=(jb == jmax - 1),
                                skip_group_check=True,
                            )
                            addPE(2.0 * (IC - o))

                        pend.append(av_unit)
                        drain(DELAY)
                        while budget_pull():
                            pass
                        t_idx += 1

                    # normalize straight from PSUM; OT written in bf16
                    csl = slice(IC * cp, IC * (cp + 1))
                    rec_A = rec_pool.tile([64, IC], f32, name="recA", tag="rec")
                    rec_B = rec_pool.tile([64, IC], f32, name="recB", tag="rec")

                    def recip_a(pO_A=pO_A, rec_A=rec_A):
                        nc.vector.reciprocal_approx_fast(out=rec_A, in_=pO_A[0:64, :])

                    def mul_a(pO_A=pO_A, rec_A=rec_A, p=p, csl=csl):
                        nc.vector.tensor_mul(OT[p][0:64, csl], pO_A[64:128, :], rec_A)

                    def recip_b(pO_B=pO_B, rec_B=rec_B):
                        nc.vector.reciprocal_approx_fast(out=rec_B, in_=pO_B[0:64, :])

                    def mul_b(pO_B=pO_B, rec_B=rec_B, p=p, csl=csl, cp=cp):
                        nc.vector.tensor_mul(OT[p][64:128, csl], pO_B[64:128, :], rec_B)
                        norm_done[(cp, p)] = True

                    pend.append(recip_a)
                    pend.append(mul_a)
                    pend.append(recip_b)
                    pend.append(mul_b)

            drain(0)
            # remaining fillers (late out-projections + pair-0 partials)
            for s in fillers:
                force_pull(s)
            # ---- epilogue: pair-1 half of the last 4 row-blocks ----
            for nb in range(12, 16):
                nsl = slice(128 * nb, 128 * (nb + 1))
                ob2 = osb_pool.tile([128, D], bf16, name="ob2", tag="osb2")
                for s in range(2):
                    po = pj_pool.tile([128, 512], f32, name="po1", tag="pj")
                    nc.tensor.matmul(
                        po,
                        lhsT=OT[1][:, nsl],
                        rhs=wo_sb[1][:, 512 * s : 512 * (s + 1)],
                        start=True,
                        stop=True,
                    )
                    off = 512 * (2 * (nb - 12) + s)
                    nc.vector.tensor_add(
                        ob2[:, 512 * s : 512 * (s + 1)], po, part[:, off : off + 512]
                    )
                nc.gpsimd.dma_start(out=outp[nsl, :], in_=ob2)

    nc.compile()
    return nc


def kernel(x, mask, Wq, Wkv, Wout, b_out):
    global _last_results
    from concourse.bass_utils import run_bass_kernel_spmd

    bf = ml_dtypes.bfloat16
    x = np.asarray(x, dtype=np.float32)
    Wq = np.asarray(Wq, dtype=np.float32)
    Wkv = np.asarray(Wkv, dtype=np.float32)
    Wout = np.asarray(Wout, dtype=np.float32)
    b_out = np.asarray(b_out, dtype=np.float32)

    if "nc" not in _cached:
        _cached["nc"] = _build_program()
    nc = _cached["nc"]

    jj, ii = np.mgrid[0:128, 0:128]
    # pexp[j, o+c] is masked (multiplied by 0) where j > c
    tri = (jj <= ii).astype(np.float32).astype(bf)

    xTs = [np.ascontiguousarray(x[b].T).astype(bf) for b in range(B)]

    in_maps = []
    for c in range(NCORES):
        b = c // 4
        h0 = HPC * (c % 4)
        in_maps.append(
            {
                "xb": xTs[b],
                "wq": np.ascontiguousarray(Wq[:, DH * h0 : DH * (h0 + HPC)]).astype(bf),
                "wk": np.ascontiguousarray(Wkv[:, DH * h0 : DH * (h0 + HPC)]).astype(bf),
                "wv": np.ascontiguousarray(Wkv[:, D + DH * h0 : D + DH * (h0 + HPC)]).astype(bf),
                "wo": np.ascontiguousarray(Wout[DH * h0 : DH * (h0 + HPC), :]).astype(bf),
                "tri": tri,
            }
        )

    res = run_bass_kernel_spmd(
        nc,
        in_maps,
        core_ids=list(range(NCORES)),
        trace=bool(int(os.environ.get("KERNEL_TRACE", "0"))),
    )
    _last_results = res
    parts = [r["outp"] for r in res.results]
    out = np.empty((B, N, D), dtype=np.float32)
    for b in range(B):
        acc = parts[4 * b].astype(np.float32).copy()
        for c in range(4 * b + 1, 4 * b + 4):
            acc += parts[c]
        out[b] = acc + b_out[None, :]
    return out
